# revision 73
# baseline (speedup 1.0000x reference)
"""Segment-reduce (min/max/mean per contiguous span) on 8 Trainium2 cores.

Sharding: pure data parallel -- core b handles batch b. Programs are
specialized at build time on the span structure (span_idxs is host data).

Per-core algorithm (v2.1, fold-bucket design):

- min/max: each span is binary-decomposed into power-of-2 chunks
  (L = sum 2^k, capped at 64), so the per-lam fold buckets carry no padding
  (only 1-token chunks pad to 2 rows). Buckets are laid out
  [lam, 4chunk, n] feature-major (partition p = d % 128, c = d // 128,
  bf16); each sub-bucket is one DMA piece and one independent
  tensor_tensor fold-tree chain (bf16 2x DVE mode, 0.52 ns/elem) on the
  DVE. (GPSIMD fold offload is plumbed but disabled: the Pool engine
  rejects TensorTensor opcodes on this target.) Big chains stop at 2-row
  remnants in a shared R2 array finished by one final TT per stat. Chunk
  partials land contiguously in bucket order; the host combines them per
  span with minimum.at/maximum.at (output-sized work). No masks, no
  scans, no per-span extraction. Fold widths are kept >= 64 elements
  (narrower DVE TTs misbehave here).
- sum/mean: TensorE matmul. lhsT = packed one-hot [128 tok, spans_in_tile]
  (fp8, ~10 cols per K-tile), rhs = x^T tile [128 tok, 512 d] (fp8),
  accumulating seg-sums in PSUM [s, d] (two banks for s 0-127 / 128-255,
  pre-zeroed by DVE). ACT scales by per-partition 1/L (activation Copy
  with scale vector) straight out of PSUM.
- spans with L <= 8 additionally get an exact bf16 fold-sum (fp8 error on
  tiny spans could breach tolerance): sum-fold over the padded rows, minus
  a host correction (lam-L)*x[start], times 1/L. Host takes mean for these
  spans from this path.

Outputs are bf16 (tolerance 2e-2); the host reassembles/permutes/casts.

Execution: each specialized program runs on its own NeuronCore via the
PJRT custom-call primitive (run_bass_via_pjrt's single-core path).
"""

import sys
import threading

sys.path.insert(0, "/opt/trn_rl_repo")

import numpy as np

B, T, D, S = 8, 4096, 512, 256
NK = T // 128  # matmul K-tiles
SUB_MAX = 10000  # max per-partition elems in one sub-bucket (DMA piece)
LEAD_N = 16  # lead sub-bucket columns for the first-issued group
GP_TARGET = 0  # fold elems (2 stats) assigned to GPSIMD (0 = GP disabled)
GP_EXTRA_LAMS = ()  # additional lam groups folded on GPSIMD
MEAN_ON_GP = False  # mean = psum * 1/L on GPSIMD instead of DVE


def _spans(span_starts):
    starts = span_starts.astype(np.int64)
    ends = np.empty_like(starts)
    ends[:-1] = starts[1:] - 1
    ends[-1] = T - 1
    return starts, ends


def _plan(starts, ends):
    """Bucket layout, sub-splitting, engine assignment, K-tile packing."""
    L = ends - starts + 1

    # Binary decomposition: each span is chunked into its power-of-2
    # components (L = sum of 2^k, capped at 64), so fold buckets carry NO
    # padding at all (except 1-token chunks padded to 2). The host combines
    # the per-chunk partials (min/max: minimum.at; small-span sums: add.at)
    # -- output-sized work.
    ps_sid, ps_off, ps_len = [], [], []
    for s in range(S):
        Ls = int(L[s])
        o = 0
        while Ls > 0:
            c = min(1 << (Ls.bit_length() - 1), 64)
            ps_sid.append(s)
            ps_off.append(o)
            ps_len.append(c)
            o += c
            Ls -= c
    ps_sid = np.array(ps_sid)
    ps_off = np.array(ps_off)
    ps_len = np.array(ps_len)
    lam = np.maximum(2, ps_len)

    groups = {}
    for l in sorted(set(lam.tolist()), reverse=True):
        idx = np.where(lam == l)[0]
        groups[l] = idx

    # GPSIMD takes the big lam-group whose 2-stat fold work (to 8-row
    # remnants) is closest to GP_TARGET. (GP custom tensor ops are not
    # supported by the axon lowering -- keep disabled until they are.)
    gp_lam = None
    best = None
    if GP_TARGET > 0:
        for l, spans in groups.items():
            if l < 16:
                continue
            work = 2 * 4 * len(spans) * (l - 8)
            score = abs(work - GP_TARGET)
            if best is None or score < best:
                best = score
                gp_lam = l

    # sub-bucket splitting; a `lead`-column first sub lets its engine start
    # folding as soon as the first (small) DMA piece lands
    def make_subs(l, spans, gp, lead=0):
        n = len(spans)
        if n == 0:
            return []
        subs = []
        i0 = 0
        if gp and n > 12:
            subs.append(spans[:8])
            i0 = 8
        elif lead and n > lead + 8:
            subs.append(spans[:lead])
            i0 = lead
        max_n = max(2, SUB_MAX // (l * 4))
        rem = n - i0
        nsub = (rem + max_n - 1) // max_n
        per = (rem + nsub - 1) // nsub if nsub else rem
        for i in range(i0, n, per):
            subs.append(spans[i : i + per])
        # n >= 8 for big subs: keeps every fold width >= 64 elements
        # (narrower DVE tensor_tensor ops misbehave on this backend)
        return [
            dict(
                lam=l,
                spans=sp,
                nreal=len(sp),
                n=max(len(sp) + (len(sp) % 2), 8 if l >= 16 else 2),
                big=(l >= 16),
                gp=gp,
            )
            for sp in subs
        ]

    gp_lams = {gp_lam} | set(GP_EXTRA_LAMS) if gp_lam else set(GP_EXTRA_LAMS)
    # the smallest-work big group is issued first -- give it a small lead sub
    big_work = {l: l * 4 * len(sp) for l, sp in groups.items() if l >= 16}
    lead_lam = min(big_work, key=big_work.get) if big_work else None
    gp_subs = []
    small_subs = []
    dve_big_subs = []
    for l, spans in groups.items():
        if l >= 16 and l in gp_lams:
            gp_subs.extend(make_subs(l, spans, True))
        elif l >= 16:
            dve_big_subs.extend(
                make_subs(l, spans, False, lead=LEAD_N if l == lead_lam else 0)
            )
        else:
            small_subs.extend(make_subs(l, spans, False))

    # APAD / DMA-piece order: GP data first, then smalls, then DVE bigs.
    order = gp_subs + small_subs + dve_big_subs
    off = 0
    for sb_ in order:
        sb_["off"] = off
        off += sb_["lam"] * 4 * sb_["n"]
    W = off

    # output columns: bigs (R2 order = their order in `order`), then smalls
    bigs = [s for s in order if s["big"]]
    smalls = [s for s in order if not s["big"]]
    NB = sum(s["n"] for s in bigs)
    NS = sum(s["n"] for s in smalls)
    SW = NB + NS
    col = 0
    for s in bigs:
        s["col"] = col  # also its R2 column offset
        col += s["n"]
    scol = 0
    for s in smalls:
        s["col"] = NB + scol
        s["s_off"] = scol
        scol += s["n"]
    perm = np.full(SW, -1, np.int64)
    for s in order:
        perm[s["col"] : s["col"] + s["nreal"]] = ps_sid[s["spans"]]
    sperm = perm[NB:]

    # DMA pieces: one per big sub; all smalls together.
    # Transfer order (= SP issue order): interleave GP/DVE data so both
    # engines start early; AT (issued by ACT) lands mid-stream.
    pieces = []
    for s in gp_subs:
        pieces.append([s])
    if smalls:
        pieces.append(list(smalls))
    for s in dve_big_subs:
        pieces.append([s])
    for i, pc in enumerate(pieces):
        for s in pc:
            s["piece"] = i
    # issue order (sim-tuned): smallest DVE big group first (earliest DVE
    # start), then GP subs + smalls, then remaining groups by elems desc
    gsz = {}
    for s in dve_big_subs:
        gsz[s["lam"]] = gsz.get(s["lam"], 0) + s["lam"] * 4 * s["n"]
    issue = []
    if gsz:
        lmin = min(gsz, key=gsz.get)
        for s in dve_big_subs:
            if s["lam"] == lmin:
                issue.append(s["piece"])
    for s in gp_subs:
        issue.append(s["piece"])
    if smalls:
        issue.append(smalls[0]["piece"])
    for s in sorted(dve_big_subs, key=lambda s: -gsz[s["lam"]]):
        issue.append(s["piece"])
    seen = set()
    issue = [i for i in issue if not (i in seen or seen.add(i))]

    # token -> span id; K-tile one-hot packing (spans are the matmul free
    # dim, so no alignment constraints)
    seg = np.searchsorted(starts, np.arange(T), side="right") - 1
    ktiles = []
    oh_off = 0
    for q in range(NK):
        s_lo = int(seg[128 * q])
        s_hi = int(seg[128 * q + 127])
        m = s_hi - s_lo + 1
        ktiles.append(dict(s_lo=s_lo, m=m, off=oh_off))
        oh_off += m
    OHW = oh_off

    return dict(
        starts=starts,
        ends=ends,
        L=L,
        lam=lam,
        seg=seg,
        ps_start=starts[ps_sid] + ps_off,
        ps_len=ps_len,
        ps_sid=ps_sid,
        order=order,
        pieces=pieces,
        issue=issue,
        at_gate=issue[min(3, len(issue) - 1)],
        gp_subs=gp_subs,
        small_subs=smalls,
        dve_big_subs=dve_big_subs,
        W=W,
        NB=NB,
        NS=NS,
        SW=SW,
        perm=perm,
        sperm=sperm,
        ktiles=ktiles,
        OHW=OHW,
    )


def _build_program(plan):
    import concourse.bass as bass
    import concourse.mybir as mybir

    f32 = mybir.dt.float32
    bf16 = mybir.dt.bfloat16
    fp8 = mybir.dt.float8e4
    Alu = mybir.AluOpType
    Act = mybir.ActivationFunctionType
    nc = bass.Bass(target_bir_lowering=False)

    W, NB, NS, SW, OHW = plan["W"], plan["NB"], plan["NS"], plan["SW"], plan["OHW"]
    ktiles = plan["ktiles"]
    pieces = plan["pieces"]
    gp_subs = plan["gp_subs"]
    smalls = plan["small_subs"]
    dve_bigs = plan["dve_big_subs"]

    APAD = nc.dram_tensor("APAD", [128, W], bf16, kind="ExternalInput")
    AT = nc.dram_tensor("AT", [128, NK * D], fp8, kind="ExternalInput")
    OH = nc.dram_tensor("OH", [128, OHW], fp8, kind="ExternalInput")
    RC = nc.dram_tensor("RC", [128, 2 * S], bf16, kind="ExternalInput")
    CORRS = nc.dram_tensor("CORRS", [128, 8 * NS], bf16, kind="ExternalInput")
    # OUT planes: [min 4*SW | max 4*SW | smean 4*NS | mean 4*S], all d-major
    O_MIN, O_MAX = 0, 4 * SW
    O_SMEAN = 8 * SW
    O_ME = 8 * SW + 4 * NS
    OUTW = O_ME + 4 * S
    OUT = nc.dram_tensor("OUT", [128, OUTW], bf16, kind="ExternalOutput")

    from contextlib import ExitStack

    with ExitStack() as ctx:
        block = ctx.enter_context(nc.Block())
        sem = lambda n: ctx.enter_context(nc.semaphore(n))
        sb = lambda n, shape, dt: ctx.enter_context(nc.sbuf_tensor(n, shape, dt))

        psems = [sem(f"p{i}_sem") for i in range(len(pieces))]
        at_sems = [sem("at0_sem"), sem("at1_sem")]
        oh_sem = sem("oh_sem")
        rc_sem = sem("rc_sem")
        cs_sem = sem("cs_sem")
        psum_sem = sem("psum_sem")
        gp_rem = [sem("gp_rem0"), sem("gp_rem1")]
        min_done = sem("min_done")
        max_done = sem("max_done")
        mean_done = sem("mean_done")
        smean_done = sem("smean_done")
        o_sem = sem("o_sem")

        APAD_sb = sb("APAD_sb", [128, W], bf16)
        AT_sb = sb("AT_sb", [128, NK * D], fp8)
        OH_sb = sb("OH_sb", [128, OHW], fp8)
        RC_sb = sb("RC_sb", [128, 2 * S], bf16)
        CORRS_sb = sb("CORRS_sb", [128, 8 * NS], bf16)
        OUT_sb = sb("OUT_sb", [128, OUTW], bf16)
        R2 = [sb(f"R2_{s}", [128, 2 * 4 * NB], bf16) for s in range(2)]
        SS = sb("SS", [128, 4 * NS], bf16)
        SS2 = sb("SS2", [128, 4 * NS], bf16)
        P0 = ctx.enter_context(nc.psum_tensor("P0", [128, 512], f32))
        P1 = ctx.enter_context(nc.psum_tensor("P1", [128, 512], f32))

        # per-engine ping-pong fold scratch (reused chain to chain)
        def pool_sizes(subs, floor):
            a = b = 0
            for s_ in subs:
                szs = []
                rows = s_["lam"] // 2
                while rows >= floor:
                    szs.append(rows * 4 * s_["n"])
                    rows //= 2
                for i, sz in enumerate(szs):
                    if i % 2 == 0:
                        a = max(a, sz)
                    else:
                        b = max(b, sz)
            return a, b

        da, db = pool_sizes(dve_bigs + smalls, 1)
        da = max(da, 8 * 4 * max((s["n"] for s in gp_subs), default=0) // 2)
        ga, gb = pool_sizes(gp_subs, 8)
        DP = [sb("dpoolA", [128, max(da, 4)], bf16), sb("dpoolB", [128, max(db, 4)], bf16)]
        GPP = [sb("gpoolA", [128, max(ga, 4)], bf16), sb("gpoolB", [128, max(gb, 4)], bf16)]
        # persistent 8-row remnants for GP subs (read later by DVE)
        R8s = {}
        for sg in range(2):
            for i, s_ in enumerate(gp_subs):
                R8s[(sg, i)] = sb(f"r8_{sg}_{i}", [128, 8 * 4 * s_["n"]], bf16)

        def bview(s_):
            return (
                APAD_sb[:, s_["off"] : s_["off"] + s_["lam"] * 4 * s_["n"]]
                .rearrange("p (j c n) -> p j c n", j=s_["lam"], c=4)
            )

        def r2v(sg):
            return R2[sg][:].rearrange("p (j c n) -> p j c n", j=2, c=4)

        def out_cols(base, width, cols, n):
            return (
                OUT_sb[:, base : base + 4 * width]
                .rearrange("p (c w) -> p c w", c=4)[:, :, cols : cols + n]
            )

        OPS = {0: Alu.min, 1: Alu.max, 2: Alu.add}

        def fold_chain(eng, sg, s_, pool, cur=None, rows=None, stop_rows=None):
            """Fold [rows,4,n] by halving. Returns last instr.

            stop_rows=2 big chains write their last level into R2 columns;
            stop_rows=8 (GP) writes into the sub's persistent R8s buffer;
            stop_rows=1 (smalls / smallsum) writes OUT / SS.
            """
            op = OPS[sg]
            n = s_["n"]
            rw = 4 * n  # row width (elements) -- rows are contiguous
            if cur is None:
                cur = APAD_sb[:, s_["off"] : s_["off"] + s_["lam"] * rw]
                rows = s_["lam"]
            last = None
            pi = 0
            while rows > stop_rows:
                h = rows // 2
                in0 = cur[:, : h * rw]
                in1 = cur[:, h * rw : 2 * h * rw]
                if h == stop_rows and stop_rows == 2:
                    dst = r2v(sg)[:, :, :, s_["col"] : s_["col"] + n]
                    in0 = in0.rearrange("p (j c n) -> p j c n", j=2, c=4)
                    in1 = in1.rearrange("p (j c n) -> p j c n", j=2, c=4)
                elif h == stop_rows and stop_rows == 8:
                    dst = R8s[(sg, s_["gpi"])][:, : h * rw]
                elif h == 1:
                    if sg == 2:
                        dst = SS[:].rearrange("p (c n) -> p c n", c=4)[
                            :, :, s_["s_off"] : s_["s_off"] + n
                        ]
                    else:
                        dst = out_cols(O_MIN if sg == 0 else O_MAX, SW, s_["col"], n)
                    in0 = in0.rearrange("p (c n) -> p c n", c=4)
                    in1 = in1.rearrange("p (c n) -> p c n", c=4)
                else:
                    dst = pool[pi % 2][:, : h * rw]
                    pi += 1
                last = eng.tensor_tensor(dst, in0, in1, op)
                cur = dst
                rows = h
            return last

        for i, s_ in enumerate(gp_subs):
            s_["gpi"] = i

        @block.sync
        def _(sy):
            for i in plan["issue"]:
                pc = pieces[i]
                lo = pc[0]["off"]
                hi = pc[-1]["off"] + pc[-1]["lam"] * 4 * pc[-1]["n"]
                sy.dma_start(APAD_sb[:, lo:hi], APAD[:, lo:hi]).then_inc(psems[i], 16)

        @block.scalar
        def _(sc):
            sc.dma_start(OH_sb[:], OH[:]).then_inc(oh_sem, 16)
            sc.dma_start(RC_sb[:], RC[:]).then_inc(rc_sem, 16)
            sc.dma_start(CORRS_sb[:], CORRS[:]).then_inc(cs_sem, 16)
            # gate AT behind the early APAD pieces (PE only needs psum late)
            sc.wait_ge(psems[plan["at_gate"]], 16)
            sc.dma_start(AT_sb[:, : 16 * D], AT[:, : 16 * D]).then_inc(at_sems[0], 16)
            sc.dma_start(AT_sb[:, 16 * D :], AT[:, 16 * D :]).then_inc(at_sems[1], 16)
            sc.wait_ge(smean_done, 1)
            sc.dma_start(
                OUT[:, O_SMEAN : O_SMEAN + 4 * NS],
                OUT_sb[:, O_SMEAN : O_SMEAN + 4 * NS],
            ).then_inc(o_sem, 16)
            sc.wait_ge(mean_done, 1)
            sc.dma_start(
                OUT[:, O_ME : O_ME + 4 * S], OUT_sb[:, O_ME : O_ME + 4 * S]
            ).then_inc(o_sem, 16)
            sc.wait_ge(min_done, 1)
            sc.dma_start(
                OUT[:, O_MIN : O_MIN + 4 * SW], OUT_sb[:, O_MIN : O_MIN + 4 * SW]
            ).then_inc(o_sem, 16)
            sc.wait_ge(max_done, 1)
            sc.dma_start(
                OUT[:, O_MAX : O_MAX + 4 * SW], OUT_sb[:, O_MAX : O_MAX + 4 * SW]
            ).then_inc(o_sem, 16)
            sc.wait_ge(o_sem, 64)

        @block.tensor
        def _(pe):
            pe.wait_ge(oh_sem, 16)
            for half in range(2):
                pe.wait_ge(at_sems[half], 16)
                for q in range(16 * half, 16 * half + 16):
                    kt = ktiles[q]
                    for c in range(4):
                        P = P0 if c < 2 else P1
                        coloff = 256 * (c % 2)
                        is_last = q == NK - 1 and c % 2 == 1
                        mm = nc.tensor.matmul(
                            P[:, coloff + kt["s_lo"] : coloff + kt["s_lo"] + kt["m"]],
                            AT_sb[:, D * q + 128 * c : D * q + 128 * (c + 1)],
                            OH_sb[:, kt["off"] : kt["off"] + kt["m"]],
                            start=(q == 0 and c % 2 == 0),
                            stop=is_last,
                            skip_group_check=True,
                        )
                        if is_last:
                            mm.then_inc(psum_sem, 1)

        def emit_mean(eng):
            # ME plane is c-major [4, S]; P0 holds chunks 0,1 / P1 chunks 2,3
            # at matching column pairs, so two wide TTs cover all four.
            eng.wait_ge(psum_sem, 2)
            eng.wait_ge(rc_sem, 16)
            eng.tensor_tensor(
                OUT_sb[:, O_ME : O_ME + 512], P0[:], RC_sb[:], Alu.mult
            )
            return eng.tensor_tensor(
                OUT_sb[:, O_ME + 512 : O_ME + 1024], P1[:], RC_sb[:], Alu.mult
            )

        @block.gpsimd
        def _(g):
            for sg in range(2):
                last = None
                for s_ in gp_subs:
                    g.wait_ge(psems[s_["piece"]], 16)
                    last = fold_chain(g, sg, s_, GPP, stop_rows=8)
                if last is not None:
                    last.then_inc(gp_rem[sg], 1)
                else:
                    g.sem_inc(gp_rem[sg], 1)
            if MEAN_ON_GP:
                emit_mean(g).then_inc(mean_done, 1)

        @block.vector
        def _(v):
            # process pieces in DMA issue order; min+max per piece so late
            # pieces don't block early work. The LAST big piece is handled
            # specially: mean runs before it and the finals interleave with
            # its min/max chains so output DMAs overlap the fold tail.
            def emit_piece(i, sgs):
                pc = pieces[i]
                if not pc[0]["big"]:
                    for sg in (0, 1, 2):
                        for s_ in pc:
                            fold_chain(v, sg, s_, DP, stop_rows=1)
                    v.wait_ge(cs_sem, 16)
                    v.tensor_tensor(
                        SS2[:], SS[:], CORRS_sb[:, : 4 * NS], Alu.subtract
                    )
                    v.tensor_tensor(
                        OUT_sb[:, O_SMEAN : O_SMEAN + 4 * NS],
                        SS2[:],
                        CORRS_sb[:, 4 * NS :],
                        Alu.mult,
                    ).then_inc(smean_done, 1)
                else:
                    for s_ in pc:
                        for sg in sgs:
                            fold_chain(v, sg, s_, DP, stop_rows=2)

            def emit_final(sg):
                if gp_subs:
                    v.wait_ge(gp_rem[sg], 1)
                    for s_ in gp_subs:
                        fold_chain(
                            v, sg, s_, DP,
                            cur=R8s[(sg, s_["gpi"])][:],
                            rows=8,
                            stop_rows=2,
                        )
                if NB:
                    base = O_MIN if sg == 0 else O_MAX
                    v.tensor_tensor(
                        out_cols(base, SW, 0, NB),
                        r2v(sg)[:, 0],
                        r2v(sg)[:, 1],
                        OPS[sg],
                    )
                v.drain()
                v.sem_inc(min_done if sg == 0 else max_done, 1)

            dve_ids = [i for i in plan["issue"] if not pieces[i][0]["gp"]]
            last_big = None
            for i in reversed(dve_ids):
                if pieces[i][0]["big"]:
                    last_big = i
                    break
            for i in dve_ids:
                if i == last_big:
                    continue
                v.wait_ge(psems[i], 16)
                emit_piece(i, (0, 1))
            if not MEAN_ON_GP:
                emit_mean(v)
                v.drain()
                v.sem_inc(mean_done, 1)
            if last_big is not None:
                v.wait_ge(psems[last_big], 16)
                emit_piece(last_big, (0,))
                emit_final(0)
                emit_piece(last_big, (1,))
                emit_final(1)
            else:
                emit_final(0)
                emit_final(1)

    return nc


def _pack_inputs(input, plans):
    import ml_dtypes

    bf16 = ml_dtypes.bfloat16
    try:
        fp8 = ml_dtypes.float8_e4m3
    except AttributeError:
        fp8 = ml_dtypes.float8_e4m3fn

    in_maps = []
    for b in range(B):
        x = input[b]  # [T, D] f32
        plan = plans[b]
        W, NS, OHW = plan["W"], plan["NS"], plan["OHW"]
        starts, L = plan["starts"], plan["L"]

        ps_start, ps_len = plan["ps_start"], plan["ps_len"]
        APAD = np.zeros((128, W), np.float32)
        for bk in plan["order"]:
            lamk, n, nreal = bk["lam"], bk["n"], bk["nreal"]
            spans = bk["spans"]
            j = np.arange(lamk)
            tok = np.where(
                j[None, :] < ps_len[spans][:, None],
                ps_start[spans][:, None] + j[None, :],
                ps_start[spans][:, None],
            )
            arr = x[tok]  # [nreal, lam, D]
            arr = arr.reshape(nreal, lamk, 4, 128).transpose(3, 1, 2, 0)
            dst = APAD[:, bk["off"] : bk["off"] + lamk * 4 * n].reshape(
                128, lamk, 4, n
            )
            dst[:, :, :, :nreal] = arr
        APAD = APAD.astype(bf16)

        AT = np.ascontiguousarray(
            x.reshape(NK, 128, D).transpose(1, 0, 2).reshape(128, NK * D)
        ).astype(fp8)

        OHm = np.zeros((128, OHW), np.float32)
        seg = plan["seg"]
        t = np.arange(128)
        for q, kt in enumerate(plan["ktiles"]):
            OHm[t, kt["off"] + seg[128 * q + t] - kt["s_lo"]] = 1.0
        OHm = OHm.astype(fp8)

        rc1 = 1.0 / L.astype(np.float32)
        RC = np.ascontiguousarray(
            np.broadcast_to(np.concatenate([rc1, rc1])[None, :], (128, 2 * S))
        ).astype(bf16)

        CORRS = np.zeros((128, 8 * NS), np.float32)
        corr = CORRS[:, : 4 * NS].reshape(128, 4, NS)
        rcs = CORRS[:, 4 * NS :].reshape(128, 4, NS)
        for bk in plan["small_subs"]:
            spans = bk["spans"]
            pad = (bk["lam"] - ps_len[spans]).astype(np.float32)
            x0 = x[ps_start[spans]]  # [nreal, D]
            cc = (pad[:, None] * x0).reshape(-1, 4, 128).transpose(2, 1, 0)
            sl = slice(bk["s_off"], bk["s_off"] + bk["nreal"])
            corr[:, :, sl] = cc
            # divide each chunk's exact sum by the ORIGINAL span length; the
            # host adds chunk partials, yielding the exact mean
            Lo = L[plan["ps_sid"][spans]].astype(np.float32)
            rcs[:, :, sl] = (1.0 / Lo)[None, None, :]
        CORRS = CORRS.astype(bf16)

        in_maps.append({"APAD": APAD, "AT": AT, "OH": OHm, "RC": RC, "CORRS": CORRS})
    return in_maps


def _unpack(res_b, plan):
    NB, NS, SW = plan["NB"], plan["NS"], plan["SW"]
    O = res_b["OUT"].astype(np.float32)
    O_MIN, O_MAX = 0, 4 * SW
    O_SMEAN = 8 * SW
    O_ME = 8 * SW + 4 * NS

    def plane(base, width):
        return (
            O[:, base : base + 4 * width]
            .reshape(128, 4, width)
            .transpose(2, 1, 0)
            .reshape(width, D)
        )

    out = np.zeros((S, 3 * D), np.float32)
    perm = plan["perm"]
    valid = perm >= 0
    # long spans were chunked into pseudo-spans: combine partial columns
    mn = np.full((S, D), np.inf, np.float32)
    mx = np.full((S, D), -np.inf, np.float32)
    np.minimum.at(mn, perm[valid], plane(O_MIN, SW)[valid])
    np.maximum.at(mx, perm[valid], plane(O_MAX, SW)[valid])
    out[:, 0:D] = mn
    out[:, D : 2 * D] = mx
    out[:, 2 * D :] = plane(O_ME, S)
    if NS:
        sperm = plan["sperm"]
        sv = sperm >= 0
        acc = np.zeros((S, D), np.float32)
        np.add.at(acc, sperm[sv], plane(O_SMEAN, NS)[sv])
        sm = plan["L"] <= 8  # exact bf16 path only for short spans
        out[sm, 2 * D :] = acc[sm]
    return out


class CoreRunner:
    """jit-once runner for one specialized program on one NeuronCore."""

    def __init__(self, nc, device, core_id):
        import jax
        import concourse.mybir as mybir
        from concourse.bass2jax import install_neuronx_cc_hook, _bass_exec_p

        install_neuronx_cc_hook()
        self.device = device
        self.core_id = core_id
        self.pid_name = (
            nc.partition_id_tensor.name if nc.partition_id_tensor is not None else None
        )
        self.in_names = []
        self.out_names = []
        out_avals = []
        self.zero_outs = []
        for alloc in nc.m.functions[0].allocations:
            if not isinstance(alloc, mybir.MemoryLocationSet):
                continue
            name = alloc.memorylocations[0].name
            if alloc.kind == "ExternalInput":
                self.in_names.append(name)
            elif alloc.kind == "ExternalOutput":
                self.out_names.append(name)
                shape = tuple(alloc.tensor_shape)
                dt = mybir.dt.np(alloc.dtype)
                out_avals.append(jax.core.ShapedArray(shape, dt))
                self.zero_outs.append(np.zeros(shape, dt))
        all_in = tuple(self.in_names + self.out_names)
        n_params = len(self.in_names)
        out_names = tuple(self.out_names)
        out_avals_t = tuple(out_avals)

        def _body(*args):
            return tuple(
                _bass_exec_p.bind(
                    *args,
                    out_avals=out_avals_t,
                    in_names=all_in,
                    out_names=out_names,
                    lowering_input_output_aliases=(),
                    sim_require_finite=False,
                    sim_require_nnan=False,
                    nc=nc,
                )
            )

        self._jit = jax.jit(
            _body, donate_argnums=tuple(range(n_params, n_params + len(out_names)))
        )

    def start(self, in_map):
        import jax

        if self.pid_name is not None:
            in_map = {**in_map, self.pid_name: np.array([[self.core_id]], np.uint32)}
        with jax.default_device(self.device):
            args = [np.asarray(in_map[n]) for n in self.in_names] + [
                z.copy() for z in self.zero_outs
            ]
            return self._jit(*args)

    def finish(self, out_arrs):
        return {n: np.asarray(a) for n, a in zip(self.out_names, out_arrs)}


_RUNNERS = None
_RUNNER_META = None
_LOCK = threading.Lock()


def _get_runners(span_idxs):
    global _RUNNERS, _RUNNER_META
    key = span_idxs.tobytes()
    with _LOCK:
        if _RUNNERS is not None and _RUNNER_META[0] == key:
            return _RUNNERS, _RUNNER_META[1]
        import jax

        devs = jax.devices()[:B]
        plans = [_plan(*_spans(span_idxs[b, :, 0].astype(np.int64))) for b in range(B)]
        runners = []
        for b in range(B):
            nc = _build_program(plans[b])
            runners.append(CoreRunner(nc, devs[b], b))
        _RUNNERS = runners
        _RUNNER_META = (key, plans)
        return runners, plans


def kernel(input, lengths, span_idxs):
    input = np.asarray(input, dtype=np.float32)
    lengths = np.asarray(lengths, dtype=np.int32)
    span_idxs = np.asarray(span_idxs, dtype=np.int32)

    runners, plans = _get_runners(span_idxs)
    in_maps = _pack_inputs(input, plans)

    pending = [None] * B

    def launch(b):
        pending[b] = runners[b].start(in_maps[b])

    threads = [threading.Thread(target=launch, args=(b,)) for b in range(B)]
    for t in threads:
        t.start()
    for t in threads:
        t.join()

    out = np.zeros((B, S, 3 * D), np.float32)
    for b in range(B):
        out[b] = _unpack(runners[b].finish(pending[b]), plans[b])

    valid = ~((span_idxs[..., 0] == 0) & (span_idxs[..., 1] == 0)) & (
        np.arange(S)[None, :] < lengths[:, None]
    )
    out[~valid] = 0.0
    return out


# revision 85
# speedup vs baseline: 1.0460x; 1.0460x over previous
"""Segment-reduce (min/max/mean per contiguous span) on 8 Trainium2 cores.

Sharding: pure data parallel -- core b handles batch b. Programs are
specialized at build time on the span structure (span_idxs is host data).

Per-core algorithm (v2.1, fold-bucket design):

- min/max: each span is binary-decomposed into power-of-2 chunks
  (L = sum 2^k, capped at 64), so the per-lam fold buckets carry no padding
  (only 1-token chunks pad to 2 rows). Buckets are laid out
  [lam, 4chunk, n] feature-major (partition p = d % 128, c = d // 128,
  bf16); each sub-bucket is one DMA piece and one independent
  tensor_tensor fold-tree chain (bf16 2x DVE mode, 0.52 ns/elem) on the
  DVE. (GPSIMD fold offload is plumbed but disabled: the Pool engine
  rejects TensorTensor opcodes on this target.) Big chains stop at 2-row
  remnants in a shared R2 array finished by one final TT per stat. Chunk
  partials land contiguously in bucket order; the host combines them per
  span with minimum.at/maximum.at (output-sized work). No masks, no
  scans, no per-span extraction. Fold widths are kept >= 64 elements
  (narrower DVE TTs misbehave here).
- sum/mean: TensorE matmul. lhsT = packed one-hot [128 tok, spans_in_tile]
  (fp8, ~10 cols per K-tile), rhs = x^T tile [128 tok, 512 d] (fp8),
  accumulating seg-sums in PSUM [s, d] (two banks for s 0-127 / 128-255,
  pre-zeroed by DVE). ACT scales by per-partition 1/L (activation Copy
  with scale vector) straight out of PSUM.
- spans with L <= 8 additionally get an exact bf16 fold-sum (fp8 error on
  tiny spans could breach tolerance): sum-fold over the padded rows, minus
  a host correction (lam-L)*x[start], times 1/L. Host takes mean for these
  spans from this path.

Outputs are bf16 (tolerance 2e-2); the host reassembles/permutes/casts.

Execution: each specialized program runs on its own NeuronCore via the
PJRT custom-call primitive (run_bass_via_pjrt's single-core path).
"""

import sys
import threading

sys.path.insert(0, "/opt/trn_rl_repo")

import numpy as np

B, T, D, S = 8, 4096, 512, 256
NK = T // 128  # matmul K-tiles
SUB_MAX = 10000  # max per-partition elems in one sub-bucket (DMA piece)
LEAD_N = 16  # lead sub-bucket columns for the first-issued group
GP_TARGET = 0  # fold elems (2 stats) assigned to GPSIMD (0 = GP disabled)
GP_EXTRA_LAMS = ()  # additional lam groups folded on GPSIMD
MEAN_ON_GP = False  # mean = psum * 1/L on GPSIMD instead of DVE


def _spans(span_starts):
    starts = span_starts.astype(np.int64)
    ends = np.empty_like(starts)
    ends[:-1] = starts[1:] - 1
    ends[-1] = T - 1
    return starts, ends


def _plan(starts, ends):
    """Bucket layout, sub-splitting, engine assignment, K-tile packing."""
    L = ends - starts + 1

    # Binary decomposition: each span is chunked into its power-of-2
    # components (L = sum of 2^k, capped at 64), so fold buckets carry NO
    # padding at all (except 1-token chunks padded to 2). The host combines
    # the per-chunk partials (min/max: minimum.at; small-span sums: add.at)
    # -- output-sized work.
    ps_sid, ps_off, ps_len = [], [], []
    for s in range(S):
        Ls = int(L[s])
        o = 0
        while Ls > 0:
            c = min(1 << (Ls.bit_length() - 1), 64)
            ps_sid.append(s)
            ps_off.append(o)
            ps_len.append(c)
            o += c
            Ls -= c
    ps_sid = np.array(ps_sid)
    ps_off = np.array(ps_off)
    ps_len = np.array(ps_len)
    lam = np.maximum(2, ps_len)

    groups = {}
    for l in sorted(set(lam.tolist()), reverse=True):
        idx = np.where(lam == l)[0]
        groups[l] = idx

    # GPSIMD takes the big lam-group whose 2-stat fold work (to 8-row
    # remnants) is closest to GP_TARGET. (GP custom tensor ops are not
    # supported by the axon lowering -- keep disabled until they are.)
    gp_lam = None
    best = None
    if GP_TARGET > 0:
        for l, spans in groups.items():
            if l < 16:
                continue
            work = 2 * 4 * len(spans) * (l - 8)
            score = abs(work - GP_TARGET)
            if best is None or score < best:
                best = score
                gp_lam = l

    # sub-bucket splitting; a `lead`-column first sub lets its engine start
    # folding as soon as the first (small) DMA piece lands
    def make_subs(l, spans, gp, lead=0):
        n = len(spans)
        if n == 0:
            return []
        subs = []
        i0 = 0
        if gp and n > 12:
            subs.append(spans[:8])
            i0 = 8
        elif lead and n > lead + 8:
            subs.append(spans[:lead])
            i0 = lead
        max_n = max(2, SUB_MAX // (l * 4))
        rem = n - i0
        nsub = (rem + max_n - 1) // max_n
        per = (rem + nsub - 1) // nsub if nsub else rem
        for i in range(i0, n, per):
            subs.append(spans[i : i + per])
        # n >= 8 for big subs: keeps every fold width >= 64 elements
        # (narrower DVE tensor_tensor ops misbehave on this backend)
        return [
            dict(
                lam=l,
                spans=sp,
                nreal=len(sp),
                n=max(len(sp) + (len(sp) % 2), 8 if l >= 16 else 2),
                big=(l >= 16),
                gp=gp,
            )
            for sp in subs
        ]

    gp_lams = {gp_lam} | set(GP_EXTRA_LAMS) if gp_lam else set(GP_EXTRA_LAMS)
    # the smallest-work big group is issued first -- give it a small lead sub
    big_work = {l: l * 4 * len(sp) for l, sp in groups.items() if l >= 16}
    lead_lam = min(big_work, key=big_work.get) if big_work else None
    gp_subs = []
    small_subs = []
    dve_big_subs = []
    for l, spans in groups.items():
        if l >= 16 and l in gp_lams:
            gp_subs.extend(make_subs(l, spans, True))
        elif l >= 16:
            dve_big_subs.extend(
                make_subs(l, spans, False, lead=LEAD_N if l == lead_lam else 0)
            )
        else:
            small_subs.extend(make_subs(l, spans, False))

    # APAD / DMA-piece order: GP data first, then smalls, then DVE bigs.
    order = gp_subs + small_subs + dve_big_subs
    off = 0
    for sb_ in order:
        sb_["off"] = off
        off += sb_["lam"] * 4 * sb_["n"]
    W = off

    # output columns: bigs (R2 order = their order in `order`), then smalls
    bigs = [s for s in order if s["big"]]
    smalls = [s for s in order if not s["big"]]
    NB = sum(s["n"] for s in bigs)
    NS = sum(s["n"] for s in smalls)
    SW = NB + NS
    col = 0
    for s in bigs:
        s["col"] = col  # also its R2 column offset
        col += s["n"]
    scol = 0
    for s in smalls:
        s["col"] = NB + scol
        s["s_off"] = scol
        scol += s["n"]
    perm = np.full(SW, -1, np.int64)
    for s in order:
        perm[s["col"] : s["col"] + s["nreal"]] = ps_sid[s["spans"]]
    sperm = perm[NB:]

    # DMA pieces: one per big sub; all smalls together.
    # Transfer order (= SP issue order): interleave GP/DVE data so both
    # engines start early; AT (issued by ACT) lands mid-stream.
    pieces = []
    for s in gp_subs:
        pieces.append([s])
    if smalls:
        pieces.append(list(smalls))
    for s in dve_big_subs:
        pieces.append([s])
    for i, pc in enumerate(pieces):
        for s in pc:
            s["piece"] = i
    # issue order (sim-tuned): big groups by ascending lam, with the smalls
    # piece inserted before the last (largest) group
    big_lams_asc = sorted({s["lam"] for s in dve_big_subs})
    issue = []
    for li, l in enumerate(big_lams_asc):
        if smalls and li == len(big_lams_asc) - 1:
            issue.append(smalls[0]["piece"])
        for s in dve_big_subs:
            if s["lam"] == l:
                issue.append(s["piece"])
    for s in gp_subs:
        issue.append(s["piece"])
    if smalls and not big_lams_asc:
        issue.append(smalls[0]["piece"])
    for i in range(len(pieces)):
        issue.append(i)  # completeness fallback: every piece must be loaded
    seen = set()
    issue = [i for i in issue if not (i in seen or seen.add(i))]

    # token -> span id; K-tile one-hot packing (spans are the matmul free
    # dim, so no alignment constraints)
    seg = np.searchsorted(starts, np.arange(T), side="right") - 1
    ktiles = []
    oh_off = 0
    for q in range(NK):
        s_lo = int(seg[128 * q])
        s_hi = int(seg[128 * q + 127])
        m = s_hi - s_lo + 1
        ktiles.append(dict(s_lo=s_lo, m=m, off=oh_off))
        oh_off += m
    OHW = oh_off

    return dict(
        starts=starts,
        ends=ends,
        L=L,
        lam=lam,
        seg=seg,
        ps_start=starts[ps_sid] + ps_off,
        ps_len=ps_len,
        ps_sid=ps_sid,
        order=order,
        pieces=pieces,
        issue=issue,
        at_gate=issue[min(3, len(issue) - 1)],
        gp_subs=gp_subs,
        small_subs=smalls,
        dve_big_subs=dve_big_subs,
        W=W,
        NB=NB,
        NS=NS,
        SW=SW,
        perm=perm,
        sperm=sperm,
        ktiles=ktiles,
        OHW=OHW,
    )


def _build_program(plan):
    import concourse.bass as bass
    import concourse.mybir as mybir

    f32 = mybir.dt.float32
    bf16 = mybir.dt.bfloat16
    fp8 = mybir.dt.float8e4
    Alu = mybir.AluOpType
    Act = mybir.ActivationFunctionType
    nc = bass.Bass(target_bir_lowering=False)

    W, NB, NS, SW, OHW = plan["W"], plan["NB"], plan["NS"], plan["SW"], plan["OHW"]
    ktiles = plan["ktiles"]
    pieces = plan["pieces"]
    gp_subs = plan["gp_subs"]
    smalls = plan["small_subs"]
    dve_bigs = plan["dve_big_subs"]

    APAD = nc.dram_tensor("APAD", [128, W], bf16, kind="ExternalInput")
    AT = nc.dram_tensor("AT", [128, NK * D], fp8, kind="ExternalInput")
    OH = nc.dram_tensor("OH", [128, OHW], fp8, kind="ExternalInput")
    RC = nc.dram_tensor("RC", [128, 2 * S], bf16, kind="ExternalInput")
    CORRS = nc.dram_tensor("CORRS", [128, 8 * NS], bf16, kind="ExternalInput")
    # OUT planes: [min 4*SW | max 4*SW | smean 4*NS | mean 4*S], all d-major
    O_MIN, O_MAX = 0, 4 * SW
    O_SMEAN = 8 * SW
    O_ME = 8 * SW + 4 * NS
    OUTW = O_ME + 4 * S
    OUT = nc.dram_tensor("OUT", [128, OUTW], bf16, kind="ExternalOutput")

    from contextlib import ExitStack

    with ExitStack() as ctx:
        block = ctx.enter_context(nc.Block())
        sem = lambda n: ctx.enter_context(nc.semaphore(n))
        sb = lambda n, shape, dt: ctx.enter_context(nc.sbuf_tensor(n, shape, dt))

        psems = [sem(f"p{i}_sem") for i in range(len(pieces))]
        at_sems = [sem("at0_sem"), sem("at1_sem")]
        oh_sem = sem("oh_sem")
        rc_sem = sem("rc_sem")
        cs_sem = sem("cs_sem")
        psum_sem = sem("psum_sem")
        gp_rem = [sem("gp_rem0"), sem("gp_rem1")]
        min_done = sem("min_done")
        max_done = sem("max_done")
        mean_done = sem("mean_done")
        smean_done = sem("smean_done")
        smm_done = sem("smm_done")
        o_sem = sem("o_sem")

        APAD_sb = sb("APAD_sb", [128, W], bf16)
        AT_sb = sb("AT_sb", [128, NK * D], fp8)
        OH_sb = sb("OH_sb", [128, OHW], fp8)
        RC_sb = sb("RC_sb", [128, 2 * S], bf16)
        CORRS_sb = sb("CORRS_sb", [128, 8 * NS], bf16)
        OUT_sb = sb("OUT_sb", [128, OUTW], bf16)
        R2 = [sb(f"R2_{s}", [128, 2 * 4 * NB], bf16) for s in range(2)]
        SS = sb("SS", [128, 4 * NS], bf16)
        SS2 = sb("SS2", [128, 4 * NS], bf16)
        P0 = ctx.enter_context(nc.psum_tensor("P0", [128, 512], f32))
        P1 = ctx.enter_context(nc.psum_tensor("P1", [128, 512], f32))

        # per-engine ping-pong fold scratch (reused chain to chain)
        def pool_sizes(subs, floor):
            a = b = 0
            for s_ in subs:
                szs = []
                rows = s_["lam"] // 2
                while rows >= floor:
                    szs.append(rows * 4 * s_["n"])
                    rows //= 2
                for i, sz in enumerate(szs):
                    if i % 2 == 0:
                        a = max(a, sz)
                    else:
                        b = max(b, sz)
            return a, b

        da, db = pool_sizes(dve_bigs + smalls, 1)
        da = max(da, 8 * 4 * max((s["n"] for s in gp_subs), default=0) // 2)
        ga, gb = pool_sizes(gp_subs, 8)
        DP = [sb("dpoolA", [128, max(da, 4)], bf16), sb("dpoolB", [128, max(db, 4)], bf16)]
        GPP = [sb("gpoolA", [128, max(ga, 4)], bf16), sb("gpoolB", [128, max(gb, 4)], bf16)]
        # persistent 8-row remnants for GP subs (read later by DVE)
        R8s = {}
        for sg in range(2):
            for i, s_ in enumerate(gp_subs):
                R8s[(sg, i)] = sb(f"r8_{sg}_{i}", [128, 8 * 4 * s_["n"]], bf16)

        def bview(s_):
            return (
                APAD_sb[:, s_["off"] : s_["off"] + s_["lam"] * 4 * s_["n"]]
                .rearrange("p (j c n) -> p j c n", j=s_["lam"], c=4)
            )

        def r2v(sg):
            return R2[sg][:].rearrange("p (j c n) -> p j c n", j=2, c=4)

        def out_cols(base, width, cols, n):
            return (
                OUT_sb[:, base : base + 4 * width]
                .rearrange("p (c w) -> p c w", c=4)[:, :, cols : cols + n]
            )

        OPS = {0: Alu.min, 1: Alu.max, 2: Alu.add}

        def fold_chain(eng, sg, s_, pool, cur=None, rows=None, stop_rows=None):
            """Fold [rows,4,n] by halving. Returns last instr.

            stop_rows=2 big chains write their last level into R2 columns;
            stop_rows=8 (GP) writes into the sub's persistent R8s buffer;
            stop_rows=1 (smalls / smallsum) writes OUT / SS.
            """
            op = OPS[sg]
            n = s_["n"]
            rw = 4 * n  # row width (elements) -- rows are contiguous
            if cur is None:
                cur = APAD_sb[:, s_["off"] : s_["off"] + s_["lam"] * rw]
                rows = s_["lam"]
            last = None
            pi = 0
            while rows > stop_rows:
                h = rows // 2
                in0 = cur[:, : h * rw]
                in1 = cur[:, h * rw : 2 * h * rw]
                if h == stop_rows and stop_rows == 2:
                    dst = r2v(sg)[:, :, :, s_["col"] : s_["col"] + n]
                    in0 = in0.rearrange("p (j c n) -> p j c n", j=2, c=4)
                    in1 = in1.rearrange("p (j c n) -> p j c n", j=2, c=4)
                elif h == stop_rows and stop_rows == 8:
                    dst = R8s[(sg, s_["gpi"])][:, : h * rw]
                elif h == 1:
                    if sg == 2:
                        dst = SS[:].rearrange("p (c n) -> p c n", c=4)[
                            :, :, s_["s_off"] : s_["s_off"] + n
                        ]
                    else:
                        dst = out_cols(O_MIN if sg == 0 else O_MAX, SW, s_["col"], n)
                    in0 = in0.rearrange("p (c n) -> p c n", c=4)
                    in1 = in1.rearrange("p (c n) -> p c n", c=4)
                else:
                    dst = pool[pi % 2][:, : h * rw]
                    pi += 1
                last = eng.tensor_tensor(dst, in0, in1, op)
                cur = dst
                rows = h
            return last

        for i, s_ in enumerate(gp_subs):
            s_["gpi"] = i

        @block.sync
        def _(sy):
            for i in plan["issue"]:
                pc = pieces[i]
                lo = pc[0]["off"]
                hi = pc[-1]["off"] + pc[-1]["lam"] * 4 * pc[-1]["n"]
                sy.dma_start(APAD_sb[:, lo:hi], APAD[:, lo:hi]).then_inc(psems[i], 16)

        @block.scalar
        def _(sc):
            sc.dma_start(OH_sb[:], OH[:]).then_inc(oh_sem, 16)
            sc.dma_start(RC_sb[:], RC[:]).then_inc(rc_sem, 16)
            sc.dma_start(CORRS_sb[:], CORRS[:]).then_inc(cs_sem, 16)
            # gate AT behind the early APAD pieces (PE only needs psum late)
            sc.wait_ge(psems[plan["at_gate"]], 16)
            sc.dma_start(AT_sb[:, : 16 * D], AT[:, : 16 * D]).then_inc(at_sems[0], 16)
            sc.dma_start(AT_sb[:, 16 * D :], AT[:, 16 * D :]).then_inc(at_sems[1], 16)
            def plane_slice(tn, base, lo, hi):
                return (
                    tn[:, base : base + 4 * SW]
                    .rearrange("p (c w) -> p c w", c=4)[:, :, lo:hi]
                )

            sc.wait_ge(smean_done, 1)
            sc.dma_start(
                OUT[:, O_SMEAN : O_SMEAN + 4 * NS],
                OUT_sb[:, O_SMEAN : O_SMEAN + 4 * NS],
            ).then_inc(o_sem, 16)
            sc.wait_ge(mean_done, 1)
            sc.dma_start(
                OUT[:, O_ME : O_ME + 4 * S], OUT_sb[:, O_ME : O_ME + 4 * S]
            ).then_inc(o_sem, 16)
            sc.wait_ge(min_done, 1)
            sc.dma_start(
                OUT[:, O_MIN : O_MIN + 4 * SW], OUT_sb[:, O_MIN : O_MIN + 4 * SW]
            ).then_inc(o_sem, 16)
            sc.wait_ge(max_done, 1)
            sc.dma_start(
                OUT[:, O_MAX : O_MAX + 4 * SW], OUT_sb[:, O_MAX : O_MAX + 4 * SW]
            ).then_inc(o_sem, 16)
            sc.wait_ge(o_sem, 64)

        @block.tensor
        def _(pe):
            pe.wait_ge(oh_sem, 16)
            for half in range(2):
                pe.wait_ge(at_sems[half], 16)
                for q in range(16 * half, 16 * half + 16):
                    kt = ktiles[q]
                    for c in range(4):
                        P = P0 if c < 2 else P1
                        coloff = 256 * (c % 2)
                        is_last = q == NK - 1 and c % 2 == 1
                        mm = nc.tensor.matmul(
                            P[:, coloff + kt["s_lo"] : coloff + kt["s_lo"] + kt["m"]],
                            AT_sb[:, D * q + 128 * c : D * q + 128 * (c + 1)],
                            OH_sb[:, kt["off"] : kt["off"] + kt["m"]],
                            start=(q == 0 and c % 2 == 0),
                            stop=is_last,
                            skip_group_check=True,
                        )
                        if is_last:
                            mm.then_inc(psum_sem, 1)

        def emit_mean(eng):
            # ME plane is c-major [4, S]; P0 holds chunks 0,1 / P1 chunks 2,3
            # at matching column pairs, so two wide TTs cover all four.
            eng.wait_ge(psum_sem, 2)
            eng.wait_ge(rc_sem, 16)
            eng.tensor_tensor(
                OUT_sb[:, O_ME : O_ME + 512], P0[:], RC_sb[:], Alu.mult
            )
            return eng.tensor_tensor(
                OUT_sb[:, O_ME + 512 : O_ME + 1024], P1[:], RC_sb[:], Alu.mult
            )

        @block.gpsimd
        def _(g):
            for sg in range(2):
                last = None
                for s_ in gp_subs:
                    g.wait_ge(psems[s_["piece"]], 16)
                    last = fold_chain(g, sg, s_, GPP, stop_rows=8)
                if last is not None:
                    last.then_inc(gp_rem[sg], 1)
                else:
                    g.sem_inc(gp_rem[sg], 1)
            if MEAN_ON_GP:
                emit_mean(g).then_inc(mean_done, 1)

        @block.vector
        def _(v):
            # process pieces in DMA issue order; min+max per piece so late
            # pieces don't block early work. The LAST big piece is handled
            # specially: mean runs before it and the finals interleave with
            # its min/max chains so output DMAs overlap the fold tail.
            def emit_piece(i, sgs):
                pc = pieces[i]
                if not pc[0]["big"]:
                    for sg in (0, 1, 2):
                        for s_ in pc:
                            fold_chain(v, sg, s_, DP, stop_rows=1)
                    v.wait_ge(cs_sem, 16)
                    v.tensor_tensor(
                        SS2[:], SS[:], CORRS_sb[:, : 4 * NS], Alu.subtract
                    )
                    v.tensor_tensor(
                        OUT_sb[:, O_SMEAN : O_SMEAN + 4 * NS],
                        SS2[:],
                        CORRS_sb[:, 4 * NS :],
                        Alu.mult,
                    ).then_inc(smean_done, 1)
                else:
                    for s_ in pc:
                        for sg in sgs:
                            fold_chain(v, sg, s_, DP, stop_rows=2)

            def emit_final(sg):
                if gp_subs:
                    v.wait_ge(gp_rem[sg], 1)
                    for s_ in gp_subs:
                        fold_chain(
                            v, sg, s_, DP,
                            cur=R8s[(sg, s_["gpi"])][:],
                            rows=8,
                            stop_rows=2,
                        )
                if NB:
                    base = O_MIN if sg == 0 else O_MAX
                    v.tensor_tensor(
                        out_cols(base, SW, 0, NB),
                        r2v(sg)[:, 0],
                        r2v(sg)[:, 1],
                        OPS[sg],
                    )
                v.drain()
                v.sem_inc(min_done if sg == 0 else max_done, 1)

            dve_ids = [i for i in plan["issue"] if not pieces[i][0]["gp"]]
            last_big = None
            for i in reversed(dve_ids):
                if pieces[i][0]["big"]:
                    last_big = i
                    break
            for i in dve_ids:
                if i == last_big:
                    continue
                v.wait_ge(psems[i], 16)
                emit_piece(i, (0, 1))
            if not MEAN_ON_GP:
                emit_mean(v)
                v.drain()
                v.sem_inc(mean_done, 1)
            if last_big is not None:
                v.wait_ge(psems[last_big], 16)
                emit_piece(last_big, (0,))
                emit_final(0)
                emit_piece(last_big, (1,))
                emit_final(1)
            else:
                emit_final(0)
                emit_final(1)

    return nc


def _pack_inputs(input, plans):
    import ml_dtypes

    bf16 = ml_dtypes.bfloat16
    try:
        fp8 = ml_dtypes.float8_e4m3
    except AttributeError:
        fp8 = ml_dtypes.float8_e4m3fn

    in_maps = []
    for b in range(B):
        x = input[b]  # [T, D] f32
        plan = plans[b]
        W, NS, OHW = plan["W"], plan["NS"], plan["OHW"]
        starts, L = plan["starts"], plan["L"]

        ps_start, ps_len = plan["ps_start"], plan["ps_len"]
        APAD = np.zeros((128, W), np.float32)
        for bk in plan["order"]:
            lamk, n, nreal = bk["lam"], bk["n"], bk["nreal"]
            spans = bk["spans"]
            j = np.arange(lamk)
            tok = np.where(
                j[None, :] < ps_len[spans][:, None],
                ps_start[spans][:, None] + j[None, :],
                ps_start[spans][:, None],
            )
            arr = x[tok]  # [nreal, lam, D]
            arr = arr.reshape(nreal, lamk, 4, 128).transpose(3, 1, 2, 0)
            dst = APAD[:, bk["off"] : bk["off"] + lamk * 4 * n].reshape(
                128, lamk, 4, n
            )
            dst[:, :, :, :nreal] = arr
        APAD = APAD.astype(bf16)

        AT = np.ascontiguousarray(
            x.reshape(NK, 128, D).transpose(1, 0, 2).reshape(128, NK * D)
        ).astype(fp8)

        OHm = np.zeros((128, OHW), np.float32)
        seg = plan["seg"]
        t = np.arange(128)
        for q, kt in enumerate(plan["ktiles"]):
            OHm[t, kt["off"] + seg[128 * q + t] - kt["s_lo"]] = 1.0
        OHm = OHm.astype(fp8)

        rc1 = 1.0 / L.astype(np.float32)
        RC = np.ascontiguousarray(
            np.broadcast_to(np.concatenate([rc1, rc1])[None, :], (128, 2 * S))
        ).astype(bf16)

        CORRS = np.zeros((128, 8 * NS), np.float32)
        corr = CORRS[:, : 4 * NS].reshape(128, 4, NS)
        rcs = CORRS[:, 4 * NS :].reshape(128, 4, NS)
        for bk in plan["small_subs"]:
            spans = bk["spans"]
            pad = (bk["lam"] - ps_len[spans]).astype(np.float32)
            x0 = x[ps_start[spans]]  # [nreal, D]
            cc = (pad[:, None] * x0).reshape(-1, 4, 128).transpose(2, 1, 0)
            sl = slice(bk["s_off"], bk["s_off"] + bk["nreal"])
            corr[:, :, sl] = cc
            # divide each chunk's exact sum by the ORIGINAL span length; the
            # host adds chunk partials, yielding the exact mean
            Lo = L[plan["ps_sid"][spans]].astype(np.float32)
            rcs[:, :, sl] = (1.0 / Lo)[None, None, :]
        CORRS = CORRS.astype(bf16)

        in_maps.append({"APAD": APAD, "AT": AT, "OH": OHm, "RC": RC, "CORRS": CORRS})
    return in_maps


def _unpack(res_b, plan):
    NB, NS, SW = plan["NB"], plan["NS"], plan["SW"]
    O = res_b["OUT"].astype(np.float32)
    O_MIN, O_MAX = 0, 4 * SW
    O_SMEAN = 8 * SW
    O_ME = 8 * SW + 4 * NS

    def plane(base, width):
        return (
            O[:, base : base + 4 * width]
            .reshape(128, 4, width)
            .transpose(2, 1, 0)
            .reshape(width, D)
        )

    out = np.zeros((S, 3 * D), np.float32)
    perm = plan["perm"]
    valid = perm >= 0
    # long spans were chunked into pseudo-spans: combine partial columns
    mn = np.full((S, D), np.inf, np.float32)
    mx = np.full((S, D), -np.inf, np.float32)
    np.minimum.at(mn, perm[valid], plane(O_MIN, SW)[valid])
    np.maximum.at(mx, perm[valid], plane(O_MAX, SW)[valid])
    out[:, 0:D] = mn
    out[:, D : 2 * D] = mx
    out[:, 2 * D :] = plane(O_ME, S)
    if NS:
        sperm = plan["sperm"]
        sv = sperm >= 0
        acc = np.zeros((S, D), np.float32)
        np.add.at(acc, sperm[sv], plane(O_SMEAN, NS)[sv])
        sm = plan["L"] <= 8  # exact bf16 path only for short spans
        out[sm, 2 * D :] = acc[sm]
    return out


class CoreRunner:
    """jit-once runner for one specialized program on one NeuronCore."""

    def __init__(self, nc, device, core_id):
        import jax
        import concourse.mybir as mybir
        from concourse.bass2jax import install_neuronx_cc_hook, _bass_exec_p

        install_neuronx_cc_hook()
        self.device = device
        self.core_id = core_id
        self.pid_name = (
            nc.partition_id_tensor.name if nc.partition_id_tensor is not None else None
        )
        self.in_names = []
        self.out_names = []
        out_avals = []
        self.zero_outs = []
        for alloc in nc.m.functions[0].allocations:
            if not isinstance(alloc, mybir.MemoryLocationSet):
                continue
            name = alloc.memorylocations[0].name
            if alloc.kind == "ExternalInput":
                self.in_names.append(name)
            elif alloc.kind == "ExternalOutput":
                self.out_names.append(name)
                shape = tuple(alloc.tensor_shape)
                dt = mybir.dt.np(alloc.dtype)
                out_avals.append(jax.core.ShapedArray(shape, dt))
                self.zero_outs.append(np.zeros(shape, dt))
        all_in = tuple(self.in_names + self.out_names)
        n_params = len(self.in_names)
        out_names = tuple(self.out_names)
        out_avals_t = tuple(out_avals)

        def _body(*args):
            return tuple(
                _bass_exec_p.bind(
                    *args,
                    out_avals=out_avals_t,
                    in_names=all_in,
                    out_names=out_names,
                    lowering_input_output_aliases=(),
                    sim_require_finite=False,
                    sim_require_nnan=False,
                    nc=nc,
                )
            )

        self._jit = jax.jit(
            _body, donate_argnums=tuple(range(n_params, n_params + len(out_names)))
        )

    def start(self, in_map):
        import jax

        if self.pid_name is not None:
            in_map = {**in_map, self.pid_name: np.array([[self.core_id]], np.uint32)}
        with jax.default_device(self.device):
            args = [np.asarray(in_map[n]) for n in self.in_names] + [
                z.copy() for z in self.zero_outs
            ]
            return self._jit(*args)

    def finish(self, out_arrs):
        return {n: np.asarray(a) for n, a in zip(self.out_names, out_arrs)}


_RUNNERS = None
_RUNNER_META = None
_LOCK = threading.Lock()


def _get_runners(span_idxs):
    global _RUNNERS, _RUNNER_META
    key = span_idxs.tobytes()
    with _LOCK:
        if _RUNNERS is not None and _RUNNER_META[0] == key:
            return _RUNNERS, _RUNNER_META[1]
        import jax

        devs = jax.devices()[:B]
        plans = [_plan(*_spans(span_idxs[b, :, 0].astype(np.int64))) for b in range(B)]
        runners = []
        for b in range(B):
            nc = _build_program(plans[b])
            runners.append(CoreRunner(nc, devs[b], b))
        _RUNNERS = runners
        _RUNNER_META = (key, plans)
        return runners, plans


def kernel(input, lengths, span_idxs):
    input = np.asarray(input, dtype=np.float32)
    lengths = np.asarray(lengths, dtype=np.int32)
    span_idxs = np.asarray(span_idxs, dtype=np.int32)

    runners, plans = _get_runners(span_idxs)
    in_maps = _pack_inputs(input, plans)

    pending = [None] * B

    def launch(b):
        pending[b] = runners[b].start(in_maps[b])

    threads = [threading.Thread(target=launch, args=(b,)) for b in range(B)]
    for t in threads:
        t.start()
    for t in threads:
        t.join()

    out = np.zeros((B, S, 3 * D), np.float32)
    for b in range(B):
        out[b] = _unpack(runners[b].finish(pending[b]), plans[b])

    valid = ~((span_idxs[..., 0] == 0) & (span_idxs[..., 1] == 0)) & (
        np.arange(S)[None, :] < lengths[:, None]
    )
    out[~valid] = 0.0
    return out


# revision 89
# speedup vs baseline: 1.0762x; 1.0288x over previous
"""Segment-reduce (min/max/mean per contiguous span) on 8 Trainium2 cores.

Sharding: pure data parallel -- core b handles batch b. Programs are
specialized at build time on the span structure (span_idxs is host data).

Per-core algorithm (v2.1, fold-bucket design):

- min/max: each span is binary-decomposed into power-of-2 chunks
  (L = sum 2^k, capped at 64), so the per-lam fold buckets carry no padding
  (only 1-token chunks pad to 2 rows). Buckets are laid out
  [lam, 4chunk, n] feature-major (partition p = d % 128, c = d // 128,
  bf16); each sub-bucket is one DMA piece and one independent
  tensor_tensor fold-tree chain (bf16 2x DVE mode, 0.52 ns/elem) on the
  DVE. (GPSIMD fold offload is plumbed but disabled: the Pool engine
  rejects TensorTensor opcodes on this target.) Big chains stop at 2-row
  remnants in a shared R2 array finished by one final TT per stat. Chunk
  partials land contiguously in bucket order; the host combines them per
  span with minimum.at/maximum.at (output-sized work). No masks, no
  scans, no per-span extraction. Fold widths are kept >= 64 elements
  (narrower DVE TTs misbehave here).
- sum/mean: TensorE matmul. lhsT = packed one-hot [128 tok, spans_in_tile]
  (fp8, ~10 cols per K-tile), rhs = x^T tile [128 tok, 512 d] (fp8),
  accumulating seg-sums in PSUM [s, d] (two banks for s 0-127 / 128-255,
  pre-zeroed by DVE). ACT scales by per-partition 1/L (activation Copy
  with scale vector) straight out of PSUM.
- spans with L <= 8 additionally get an exact bf16 fold-sum (fp8 error on
  tiny spans could breach tolerance): sum-fold over the padded rows, minus
  a host correction (lam-L)*x[start], times 1/L. Host takes mean for these
  spans from this path.

Outputs are bf16 (tolerance 2e-2); the host reassembles/permutes/casts.

Execution: each specialized program runs on its own NeuronCore via the
PJRT custom-call primitive (run_bass_via_pjrt's single-core path).
"""

import sys
import threading

sys.path.insert(0, "/opt/trn_rl_repo")

import numpy as np

B, T, D, S = 8, 4096, 512, 256
NK = T // 128  # matmul K-tiles
SUB_MAX = 10000  # max per-partition elems in one sub-bucket (DMA piece)
LEAD_N = 16  # lead sub-bucket columns for the first-issued group
GP_TARGET = 0  # fold elems (2 stats) assigned to GPSIMD (0 = GP disabled)
GP_EXTRA_LAMS = ()  # additional lam groups folded on GPSIMD
MEAN_ON_GP = False  # mean = psum * 1/L on GPSIMD instead of DVE


def _spans(span_starts):
    starts = span_starts.astype(np.int64)
    ends = np.empty_like(starts)
    ends[:-1] = starts[1:] - 1
    ends[-1] = T - 1
    return starts, ends


def _plan(starts, ends):
    """Bucket layout, sub-splitting, engine assignment, K-tile packing."""
    L = ends - starts + 1

    # Binary decomposition: each span is chunked into its power-of-2
    # components (L = sum of 2^k, capped at 64), so fold buckets carry NO
    # padding at all (except 1-token chunks padded to 2). The host combines
    # the per-chunk partials (min/max: minimum.at; small-span sums: add.at)
    # -- output-sized work.
    ps_sid, ps_off, ps_len = [], [], []
    for s in range(S):
        Ls = int(L[s])
        o = 0
        while Ls > 0:
            c = min(1 << (Ls.bit_length() - 1), 64)
            ps_sid.append(s)
            ps_off.append(o)
            ps_len.append(c)
            o += c
            Ls -= c
    ps_sid = np.array(ps_sid)
    ps_off = np.array(ps_off)
    ps_len = np.array(ps_len)
    lam = np.maximum(2, ps_len)

    groups = {}
    for l in sorted(set(lam.tolist()), reverse=True):
        idx = np.where(lam == l)[0]
        groups[l] = idx

    # GPSIMD takes the big lam-group whose 2-stat fold work (to 8-row
    # remnants) is closest to GP_TARGET. (GP custom tensor ops are not
    # supported by the axon lowering -- keep disabled until they are.)
    gp_lam = None
    best = None
    if GP_TARGET > 0:
        for l, spans in groups.items():
            if l < 16:
                continue
            work = 2 * 4 * len(spans) * (l - 8)
            score = abs(work - GP_TARGET)
            if best is None or score < best:
                best = score
                gp_lam = l

    # sub-bucket splitting; a `lead`-column first sub lets its engine start
    # folding as soon as the first (small) DMA piece lands
    def make_subs(l, spans, gp, lead=0):
        n = len(spans)
        if n == 0:
            return []
        subs = []
        i0 = 0
        if gp and n > 12:
            subs.append(spans[:8])
            i0 = 8
        elif lead and n > lead + 8:
            subs.append(spans[:lead])
            i0 = lead
        max_n = max(2, SUB_MAX // (l * 4))
        rem = n - i0
        nsub = (rem + max_n - 1) // max_n
        per = (rem + nsub - 1) // nsub if nsub else rem
        for i in range(i0, n, per):
            subs.append(spans[i : i + per])
        # n >= 8 for big subs: keeps every fold width >= 64 elements
        # (narrower DVE tensor_tensor ops misbehave on this backend)
        return [
            dict(
                lam=l,
                spans=sp,
                nreal=len(sp),
                n=max(len(sp) + (len(sp) % 2), 8 if l >= 16 else 2),
                big=(l >= 16),
                gp=gp,
            )
            for sp in subs
        ]

    gp_lams = {gp_lam} | set(GP_EXTRA_LAMS) if gp_lam else set(GP_EXTRA_LAMS)
    # the smallest-work big group is issued first -- give it a small lead sub
    big_work = {l: l * 4 * len(sp) for l, sp in groups.items() if l >= 16}
    lead_lam = min(big_work, key=big_work.get) if big_work else None
    gp_subs = []
    small_subs = []
    dve_big_subs = []
    for l, spans in groups.items():
        if l >= 16 and l in gp_lams:
            gp_subs.extend(make_subs(l, spans, True))
        elif l >= 16:
            dve_big_subs.extend(
                make_subs(l, spans, False, lead=LEAD_N if l == lead_lam else 0)
            )
        else:
            small_subs.extend(make_subs(l, spans, False))

    # APAD / DMA-piece order: GP data first, then smalls, then DVE bigs.
    order = gp_subs + small_subs + dve_big_subs
    off = 0
    for sb_ in order:
        sb_["off"] = off
        off += sb_["lam"] * 4 * sb_["n"]
    W = off

    # output columns: bigs (R2 order = their order in `order`), then smalls
    bigs = [s for s in order if s["big"]]
    smalls = [s for s in order if not s["big"]]
    NB = sum(s["n"] for s in bigs)
    NS = sum(s["n"] for s in smalls)
    SW = NB + NS
    col = 0
    for s in bigs:
        s["col"] = col  # also its R2 column offset
        col += s["n"]
    scol = 0
    for s in smalls:
        s["col"] = NB + scol
        s["s_off"] = scol
        scol += s["n"]
    perm = np.full(SW, -1, np.int64)
    for s in order:
        perm[s["col"] : s["col"] + s["nreal"]] = ps_sid[s["spans"]]
    sperm = perm[NB:]

    # DMA pieces: one per big sub; all smalls together.
    # Transfer order (= SP issue order): interleave GP/DVE data so both
    # engines start early; AT (issued by ACT) lands mid-stream.
    pieces = []
    for s in gp_subs:
        pieces.append([s])
    if smalls:
        pieces.append(list(smalls))
    for s in dve_big_subs:
        pieces.append([s])
    for i, pc in enumerate(pieces):
        for s in pc:
            s["piece"] = i
    # issue order (sim-tuned): big groups by ascending lam, with the smalls
    # piece inserted before the last (largest) group
    big_lams_asc = sorted({s["lam"] for s in dve_big_subs})
    issue = []
    for li, l in enumerate(big_lams_asc):
        if smalls and li == len(big_lams_asc) - 1:
            issue.append(smalls[0]["piece"])
        for s in dve_big_subs:
            if s["lam"] == l:
                issue.append(s["piece"])
    for s in gp_subs:
        issue.append(s["piece"])
    if smalls and not big_lams_asc:
        issue.append(smalls[0]["piece"])
    for i in range(len(pieces)):
        issue.append(i)  # completeness fallback: every piece must be loaded
    seen = set()
    issue = [i for i in issue if not (i in seen or seen.add(i))]

    # token -> span id; K-tile one-hot packing (spans are the matmul free
    # dim, so no alignment constraints)
    seg = np.searchsorted(starts, np.arange(T), side="right") - 1
    ktiles = []
    oh_off = 0
    for q in range(NK):
        s_lo = int(seg[128 * q])
        s_hi = int(seg[128 * q + 127])
        m = s_hi - s_lo + 1
        ktiles.append(dict(s_lo=s_lo, m=m, off=oh_off))
        oh_off += m
    OHW = oh_off

    return dict(
        starts=starts,
        ends=ends,
        L=L,
        lam=lam,
        seg=seg,
        ps_start=starts[ps_sid] + ps_off,
        ps_len=ps_len,
        ps_sid=ps_sid,
        order=order,
        pieces=pieces,
        issue=issue,
        at_gate=issue[min(3, len(issue) - 1)],
        gp_subs=gp_subs,
        small_subs=smalls,
        dve_big_subs=dve_big_subs,
        W=W,
        NB=NB,
        NS=NS,
        SW=SW,
        perm=perm,
        sperm=sperm,
        ktiles=ktiles,
        OHW=OHW,
    )


def _build_program(plan):
    import concourse.bass as bass
    import concourse.mybir as mybir

    f32 = mybir.dt.float32
    bf16 = mybir.dt.bfloat16
    fp8 = mybir.dt.float8e4
    Alu = mybir.AluOpType
    Act = mybir.ActivationFunctionType
    nc = bass.Bass(target_bir_lowering=False)

    W, NB, NS, SW, OHW = plan["W"], plan["NB"], plan["NS"], plan["SW"], plan["OHW"]
    ktiles = plan["ktiles"]
    pieces = plan["pieces"]
    gp_subs = plan["gp_subs"]
    smalls = plan["small_subs"]
    dve_bigs = plan["dve_big_subs"]

    APAD = nc.dram_tensor("APAD", [128, W], bf16, kind="ExternalInput")
    AT = nc.dram_tensor("AT", [128, NK * D], fp8, kind="ExternalInput")
    OH = nc.dram_tensor("OH", [128, OHW], fp8, kind="ExternalInput")
    RC = nc.dram_tensor("RC", [128, 2 * S], bf16, kind="ExternalInput")
    CORRS = nc.dram_tensor("CORRS", [128, 8 * NS], bf16, kind="ExternalInput")
    # OUT planes: [min 4*SW | max 4*SW | smean 4*NS | mean 4*S], all d-major
    O_MIN, O_MAX = 0, 4 * SW
    O_SMEAN = 8 * SW
    O_ME = 8 * SW + 4 * NS
    OUTW = O_ME + 4 * S
    OUT = nc.dram_tensor("OUT", [128, OUTW], bf16, kind="ExternalOutput")

    from contextlib import ExitStack

    with ExitStack() as ctx:
        block = ctx.enter_context(nc.Block())
        sem = lambda n: ctx.enter_context(nc.semaphore(n))
        sb = lambda n, shape, dt: ctx.enter_context(nc.sbuf_tensor(n, shape, dt))

        psems = [sem(f"p{i}_sem") for i in range(len(pieces))]
        at_sems = [sem("at0_sem"), sem("at1_sem")]
        oh_sem = sem("oh_sem")
        rc_sem = sem("rc_sem")
        cs_sem = sem("cs_sem")
        psum_sem = sem("psum_sem")
        gp_rem = [sem("gp_rem0"), sem("gp_rem1")]
        min_done = sem("min_done")
        max_done = sem("max_done")
        mean_done = sem("mean_done")
        smean_done = sem("smean_done")
        smm_done = sem("smm_done")
        o_sem = sem("o_sem")

        APAD_sb = sb("APAD_sb", [128, W], bf16)
        AT_sb = sb("AT_sb", [128, NK * D], fp8)
        OH_sb = sb("OH_sb", [128, OHW], fp8)
        RC_sb = sb("RC_sb", [128, 2 * S], bf16)
        CORRS_sb = sb("CORRS_sb", [128, 8 * NS], bf16)
        OUT_sb = sb("OUT_sb", [128, OUTW], bf16)
        R2 = [sb(f"R2_{s}", [128, 2 * 4 * NB], bf16) for s in range(2)]
        SS = sb("SS", [128, 4 * NS], bf16)
        SS2 = sb("SS2", [128, 4 * NS], bf16)
        P0 = ctx.enter_context(nc.psum_tensor("P0", [128, 512], f32))
        P1 = ctx.enter_context(nc.psum_tensor("P1", [128, 512], f32))

        # per-engine ping-pong fold scratch (reused chain to chain)
        def pool_sizes(subs, floor):
            a = b = 0
            for s_ in subs:
                szs = []
                rows = s_["lam"] // 2
                while rows >= floor:
                    szs.append(rows * 4 * s_["n"])
                    rows //= 2
                for i, sz in enumerate(szs):
                    if i % 2 == 0:
                        a = max(a, sz)
                    else:
                        b = max(b, sz)
            return a, b

        da, db = pool_sizes(dve_bigs + smalls, 1)
        da = max(da, 8 * 4 * max((s["n"] for s in gp_subs), default=0) // 2)
        ga, gb = pool_sizes(gp_subs, 8)
        DP = [sb("dpoolA", [128, max(da, 4)], bf16), sb("dpoolB", [128, max(db, 4)], bf16)]
        GPP = [sb("gpoolA", [128, max(ga, 4)], bf16), sb("gpoolB", [128, max(gb, 4)], bf16)]
        # persistent 8-row remnants for GP subs (read later by DVE)
        R8s = {}
        for sg in range(2):
            for i, s_ in enumerate(gp_subs):
                R8s[(sg, i)] = sb(f"r8_{sg}_{i}", [128, 8 * 4 * s_["n"]], bf16)

        def bview(s_):
            return (
                APAD_sb[:, s_["off"] : s_["off"] + s_["lam"] * 4 * s_["n"]]
                .rearrange("p (j c n) -> p j c n", j=s_["lam"], c=4)
            )

        def r2v(sg):
            return R2[sg][:].rearrange("p (j c n) -> p j c n", j=2, c=4)

        def out_cols(base, width, cols, n):
            return (
                OUT_sb[:, base : base + 4 * width]
                .rearrange("p (c w) -> p c w", c=4)[:, :, cols : cols + n]
            )

        OPS = {0: Alu.min, 1: Alu.max, 2: Alu.add}

        def fold_chain(eng, sg, s_, pool, cur=None, rows=None, stop_rows=None):
            """Fold [rows,4,n] by halving. Returns last instr.

            stop_rows=2 big chains write their last level into R2 columns;
            stop_rows=8 (GP) writes into the sub's persistent R8s buffer;
            stop_rows=1 (smalls / smallsum) writes OUT / SS.
            """
            op = OPS[sg]
            n = s_["n"]
            rw = 4 * n  # row width (elements) -- rows are contiguous
            if cur is None:
                cur = APAD_sb[:, s_["off"] : s_["off"] + s_["lam"] * rw]
                rows = s_["lam"]
            last = None
            pi = 0
            while rows > stop_rows:
                h = rows // 2
                in0 = cur[:, : h * rw]
                in1 = cur[:, h * rw : 2 * h * rw]
                if h == stop_rows and stop_rows == 2:
                    dst = r2v(sg)[:, :, :, s_["col"] : s_["col"] + n]
                    in0 = in0.rearrange("p (j c n) -> p j c n", j=2, c=4)
                    in1 = in1.rearrange("p (j c n) -> p j c n", j=2, c=4)
                elif h == stop_rows and stop_rows == 8:
                    dst = R8s[(sg, s_["gpi"])][:, : h * rw]
                elif h == 1:
                    if sg == 2:
                        dst = SS[:].rearrange("p (c n) -> p c n", c=4)[
                            :, :, s_["s_off"] : s_["s_off"] + n
                        ]
                    else:
                        dst = out_cols(O_MIN if sg == 0 else O_MAX, SW, s_["col"], n)
                    in0 = in0.rearrange("p (c n) -> p c n", c=4)
                    in1 = in1.rearrange("p (c n) -> p c n", c=4)
                else:
                    dst = pool[pi % 2][:, : h * rw]
                    pi += 1
                last = eng.tensor_tensor(dst, in0, in1, op)
                cur = dst
                rows = h
            return last

        for i, s_ in enumerate(gp_subs):
            s_["gpi"] = i

        @block.sync
        def _(sy):
            for i in plan["issue"]:
                pc = pieces[i]
                lo = pc[0]["off"]
                hi = pc[-1]["off"] + pc[-1]["lam"] * 4 * pc[-1]["n"]
                sy.dma_start(APAD_sb[:, lo:hi], APAD[:, lo:hi]).then_inc(psems[i], 16)

        @block.scalar
        def _(sc):
            sc.dma_start(OH_sb[:], OH[:]).then_inc(oh_sem, 16)
            sc.dma_start(RC_sb[:], RC[:]).then_inc(rc_sem, 16)
            sc.dma_start(CORRS_sb[:], CORRS[:]).then_inc(cs_sem, 16)
            # gate AT behind the early APAD pieces (PE only needs psum late)
            sc.wait_ge(psems[plan["at_gate"]], 16)
            sc.dma_start(AT_sb[:, : 16 * D], AT[:, : 16 * D]).then_inc(at_sems[0], 16)
            sc.dma_start(AT_sb[:, 16 * D :], AT[:, 16 * D :]).then_inc(at_sems[1], 16)
            def plane_slice(tn, base, lo, hi):
                return (
                    tn[:, base : base + 4 * SW]
                    .rearrange("p (c w) -> p c w", c=4)[:, :, lo:hi]
                )

            # smalls columns of the min/max planes complete early -- ship
            # them while the big-bucket folds are still running
            sc.wait_ge(smm_done, 1)
            sc.dma_start(
                plane_slice(OUT, O_MIN, NB, SW), plane_slice(OUT_sb, O_MIN, NB, SW)
            ).then_inc(o_sem, 16)
            sc.dma_start(
                plane_slice(OUT, O_MAX, NB, SW), plane_slice(OUT_sb, O_MAX, NB, SW)
            ).then_inc(o_sem, 16)
            sc.wait_ge(smean_done, 1)
            sc.dma_start(
                OUT[:, O_SMEAN : O_SMEAN + 4 * NS],
                OUT_sb[:, O_SMEAN : O_SMEAN + 4 * NS],
            ).then_inc(o_sem, 16)
            sc.wait_ge(min_done, 1)
            sc.dma_start(
                plane_slice(OUT, O_MIN, 0, NB), plane_slice(OUT_sb, O_MIN, 0, NB)
            ).then_inc(o_sem, 16)
            sc.wait_ge(mean_done, 1)
            sc.dma_start(
                OUT[:, O_ME : O_ME + 4 * S], OUT_sb[:, O_ME : O_ME + 4 * S]
            ).then_inc(o_sem, 16)
            sc.wait_ge(max_done, 1)
            sc.dma_start(
                plane_slice(OUT, O_MAX, 0, NB), plane_slice(OUT_sb, O_MAX, 0, NB)
            ).then_inc(o_sem, 16)
            sc.wait_ge(o_sem, 96)

        @block.tensor
        def _(pe):
            pe.wait_ge(oh_sem, 16)
            for half in range(2):
                pe.wait_ge(at_sems[half], 16)
                for q in range(16 * half, 16 * half + 16):
                    kt = ktiles[q]
                    for c in range(4):
                        P = P0 if c < 2 else P1
                        coloff = 256 * (c % 2)
                        is_last = q == NK - 1 and c % 2 == 1
                        mm = nc.tensor.matmul(
                            P[:, coloff + kt["s_lo"] : coloff + kt["s_lo"] + kt["m"]],
                            AT_sb[:, D * q + 128 * c : D * q + 128 * (c + 1)],
                            OH_sb[:, kt["off"] : kt["off"] + kt["m"]],
                            start=(q == 0 and c % 2 == 0),
                            stop=is_last,
                            skip_group_check=True,
                        )
                        if is_last:
                            mm.then_inc(psum_sem, 1)

        def emit_mean(eng):
            # ME plane is c-major [4, S]; P0 holds chunks 0,1 / P1 chunks 2,3
            # at matching column pairs, so two wide TTs cover all four.
            eng.wait_ge(psum_sem, 2)
            eng.wait_ge(rc_sem, 16)
            eng.tensor_tensor(
                OUT_sb[:, O_ME : O_ME + 512], P0[:], RC_sb[:], Alu.mult
            )
            return eng.tensor_tensor(
                OUT_sb[:, O_ME + 512 : O_ME + 1024], P1[:], RC_sb[:], Alu.mult
            )

        @block.gpsimd
        def _(g):
            for sg in range(2):
                last = None
                for s_ in gp_subs:
                    g.wait_ge(psems[s_["piece"]], 16)
                    last = fold_chain(g, sg, s_, GPP, stop_rows=8)
                if last is not None:
                    last.then_inc(gp_rem[sg], 1)
                else:
                    g.sem_inc(gp_rem[sg], 1)
            if MEAN_ON_GP:
                emit_mean(g).then_inc(mean_done, 1)

        @block.vector
        def _(v):
            # process pieces in DMA issue order; min+max per piece so late
            # pieces don't block early work. The LAST big piece is handled
            # specially: mean runs before it and the finals interleave with
            # its min/max chains so output DMAs overlap the fold tail.
            def emit_piece(i, sgs):
                pc = pieces[i]
                if not pc[0]["big"]:
                    for sg in (0, 1, 2):
                        last = None
                        for s_ in pc:
                            last = fold_chain(v, sg, s_, DP, stop_rows=1)
                        if sg == 1 and last is not None:
                            last.then_inc(smm_done, 1)
                    v.wait_ge(cs_sem, 16)
                    v.tensor_tensor(
                        SS2[:], SS[:], CORRS_sb[:, : 4 * NS], Alu.subtract
                    )
                    v.tensor_tensor(
                        OUT_sb[:, O_SMEAN : O_SMEAN + 4 * NS],
                        SS2[:],
                        CORRS_sb[:, 4 * NS :],
                        Alu.mult,
                    ).then_inc(smean_done, 1)
                else:
                    for s_ in pc:
                        for sg in sgs:
                            fold_chain(v, sg, s_, DP, stop_rows=2)

            def emit_final(sg):
                if gp_subs:
                    v.wait_ge(gp_rem[sg], 1)
                    for s_ in gp_subs:
                        fold_chain(
                            v, sg, s_, DP,
                            cur=R8s[(sg, s_["gpi"])][:],
                            rows=8,
                            stop_rows=2,
                        )
                if NB:
                    base = O_MIN if sg == 0 else O_MAX
                    v.tensor_tensor(
                        out_cols(base, SW, 0, NB),
                        r2v(sg)[:, 0],
                        r2v(sg)[:, 1],
                        OPS[sg],
                    )
                v.drain()
                v.sem_inc(min_done if sg == 0 else max_done, 1)

            dve_ids = [i for i in plan["issue"] if not pieces[i][0]["gp"]]
            last_big = None
            for i in reversed(dve_ids):
                if pieces[i][0]["big"]:
                    last_big = i
                    break
            for i in dve_ids:
                if i == last_big:
                    continue
                v.wait_ge(psems[i], 16)
                emit_piece(i, (0, 1))
            if not MEAN_ON_GP:
                emit_mean(v)
                v.drain()
                v.sem_inc(mean_done, 1)
            if last_big is not None:
                v.wait_ge(psems[last_big], 16)
                emit_piece(last_big, (0,))
                emit_final(0)
                emit_piece(last_big, (1,))
                emit_final(1)
            else:
                emit_final(0)
                emit_final(1)

    return nc


def _pack_inputs(input, plans):
    import ml_dtypes

    bf16 = ml_dtypes.bfloat16
    try:
        fp8 = ml_dtypes.float8_e4m3
    except AttributeError:
        fp8 = ml_dtypes.float8_e4m3fn

    in_maps = []
    for b in range(B):
        x = input[b]  # [T, D] f32
        plan = plans[b]
        W, NS, OHW = plan["W"], plan["NS"], plan["OHW"]
        starts, L = plan["starts"], plan["L"]

        ps_start, ps_len = plan["ps_start"], plan["ps_len"]
        APAD = np.zeros((128, W), np.float32)
        for bk in plan["order"]:
            lamk, n, nreal = bk["lam"], bk["n"], bk["nreal"]
            spans = bk["spans"]
            j = np.arange(lamk)
            tok = np.where(
                j[None, :] < ps_len[spans][:, None],
                ps_start[spans][:, None] + j[None, :],
                ps_start[spans][:, None],
            )
            arr = x[tok]  # [nreal, lam, D]
            arr = arr.reshape(nreal, lamk, 4, 128).transpose(3, 1, 2, 0)
            dst = APAD[:, bk["off"] : bk["off"] + lamk * 4 * n].reshape(
                128, lamk, 4, n
            )
            dst[:, :, :, :nreal] = arr
        APAD = APAD.astype(bf16)

        AT = np.ascontiguousarray(
            x.reshape(NK, 128, D).transpose(1, 0, 2).reshape(128, NK * D)
        ).astype(fp8)

        OHm = np.zeros((128, OHW), np.float32)
        seg = plan["seg"]
        t = np.arange(128)
        for q, kt in enumerate(plan["ktiles"]):
            OHm[t, kt["off"] + seg[128 * q + t] - kt["s_lo"]] = 1.0
        OHm = OHm.astype(fp8)

        rc1 = 1.0 / L.astype(np.float32)
        RC = np.ascontiguousarray(
            np.broadcast_to(np.concatenate([rc1, rc1])[None, :], (128, 2 * S))
        ).astype(bf16)

        CORRS = np.zeros((128, 8 * NS), np.float32)
        corr = CORRS[:, : 4 * NS].reshape(128, 4, NS)
        rcs = CORRS[:, 4 * NS :].reshape(128, 4, NS)
        for bk in plan["small_subs"]:
            spans = bk["spans"]
            pad = (bk["lam"] - ps_len[spans]).astype(np.float32)
            x0 = x[ps_start[spans]]  # [nreal, D]
            cc = (pad[:, None] * x0).reshape(-1, 4, 128).transpose(2, 1, 0)
            sl = slice(bk["s_off"], bk["s_off"] + bk["nreal"])
            corr[:, :, sl] = cc
            # divide each chunk's exact sum by the ORIGINAL span length; the
            # host adds chunk partials, yielding the exact mean
            Lo = L[plan["ps_sid"][spans]].astype(np.float32)
            rcs[:, :, sl] = (1.0 / Lo)[None, None, :]
        CORRS = CORRS.astype(bf16)

        in_maps.append({"APAD": APAD, "AT": AT, "OH": OHm, "RC": RC, "CORRS": CORRS})
    return in_maps


def _unpack(res_b, plan):
    NB, NS, SW = plan["NB"], plan["NS"], plan["SW"]
    O = res_b["OUT"].astype(np.float32)
    O_MIN, O_MAX = 0, 4 * SW
    O_SMEAN = 8 * SW
    O_ME = 8 * SW + 4 * NS

    def plane(base, width):
        return (
            O[:, base : base + 4 * width]
            .reshape(128, 4, width)
            .transpose(2, 1, 0)
            .reshape(width, D)
        )

    out = np.zeros((S, 3 * D), np.float32)
    perm = plan["perm"]
    valid = perm >= 0
    # long spans were chunked into pseudo-spans: combine partial columns
    mn = np.full((S, D), np.inf, np.float32)
    mx = np.full((S, D), -np.inf, np.float32)
    np.minimum.at(mn, perm[valid], plane(O_MIN, SW)[valid])
    np.maximum.at(mx, perm[valid], plane(O_MAX, SW)[valid])
    out[:, 0:D] = mn
    out[:, D : 2 * D] = mx
    out[:, 2 * D :] = plane(O_ME, S)
    if NS:
        sperm = plan["sperm"]
        sv = sperm >= 0
        acc = np.zeros((S, D), np.float32)
        np.add.at(acc, sperm[sv], plane(O_SMEAN, NS)[sv])
        sm = plan["L"] <= 8  # exact bf16 path only for short spans
        out[sm, 2 * D :] = acc[sm]
    return out


class CoreRunner:
    """jit-once runner for one specialized program on one NeuronCore."""

    def __init__(self, nc, device, core_id):
        import jax
        import concourse.mybir as mybir
        from concourse.bass2jax import install_neuronx_cc_hook, _bass_exec_p

        install_neuronx_cc_hook()
        self.device = device
        self.core_id = core_id
        self.pid_name = (
            nc.partition_id_tensor.name if nc.partition_id_tensor is not None else None
        )
        self.in_names = []
        self.out_names = []
        out_avals = []
        self.zero_outs = []
        for alloc in nc.m.functions[0].allocations:
            if not isinstance(alloc, mybir.MemoryLocationSet):
                continue
            name = alloc.memorylocations[0].name
            if alloc.kind == "ExternalInput":
                self.in_names.append(name)
            elif alloc.kind == "ExternalOutput":
                self.out_names.append(name)
                shape = tuple(alloc.tensor_shape)
                dt = mybir.dt.np(alloc.dtype)
                out_avals.append(jax.core.ShapedArray(shape, dt))
                self.zero_outs.append(np.zeros(shape, dt))
        all_in = tuple(self.in_names + self.out_names)
        n_params = len(self.in_names)
        out_names = tuple(self.out_names)
        out_avals_t = tuple(out_avals)

        def _body(*args):
            return tuple(
                _bass_exec_p.bind(
                    *args,
                    out_avals=out_avals_t,
                    in_names=all_in,
                    out_names=out_names,
                    lowering_input_output_aliases=(),
                    sim_require_finite=False,
                    sim_require_nnan=False,
                    nc=nc,
                )
            )

        self._jit = jax.jit(
            _body, donate_argnums=tuple(range(n_params, n_params + len(out_names)))
        )

    def start(self, in_map):
        import jax

        if self.pid_name is not None:
            in_map = {**in_map, self.pid_name: np.array([[self.core_id]], np.uint32)}
        with jax.default_device(self.device):
            args = [np.asarray(in_map[n]) for n in self.in_names] + [
                z.copy() for z in self.zero_outs
            ]
            return self._jit(*args)

    def finish(self, out_arrs):
        return {n: np.asarray(a) for n, a in zip(self.out_names, out_arrs)}


_RUNNERS = None
_RUNNER_META = None
_LOCK = threading.Lock()


def _get_runners(span_idxs):
    global _RUNNERS, _RUNNER_META
    key = span_idxs.tobytes()
    with _LOCK:
        if _RUNNERS is not None and _RUNNER_META[0] == key:
            return _RUNNERS, _RUNNER_META[1]
        import jax

        devs = jax.devices()[:B]
        plans = [_plan(*_spans(span_idxs[b, :, 0].astype(np.int64))) for b in range(B)]
        runners = []
        for b in range(B):
            nc = _build_program(plans[b])
            runners.append(CoreRunner(nc, devs[b], b))
        _RUNNERS = runners
        _RUNNER_META = (key, plans)
        return runners, plans


def kernel(input, lengths, span_idxs):
    input = np.asarray(input, dtype=np.float32)
    lengths = np.asarray(lengths, dtype=np.int32)
    span_idxs = np.asarray(span_idxs, dtype=np.int32)

    runners, plans = _get_runners(span_idxs)
    in_maps = _pack_inputs(input, plans)

    pending = [None] * B

    def launch(b):
        pending[b] = runners[b].start(in_maps[b])

    threads = [threading.Thread(target=launch, args=(b,)) for b in range(B)]
    for t in threads:
        t.start()
    for t in threads:
        t.join()

    out = np.zeros((B, S, 3 * D), np.float32)
    for b in range(B):
        out[b] = _unpack(runners[b].finish(pending[b]), plans[b])

    valid = ~((span_idxs[..., 0] == 0) & (span_idxs[..., 1] == 0)) & (
        np.arange(S)[None, :] < lengths[:, None]
    )
    out[~valid] = 0.0
    return out
